# revision 1
# baseline (speedup 1.0000x reference)
"""Trainium2 Bass kernel for the EntropyBottleneckLatticeFlow problem.

Computes, for inputs [2048, 32] and noise [256, 32]:
    z = inputs[b] - noise[n]  for all (b, n)            -> 524288 rows x 32
    logprob = 5x RealNVP coupling flows (4 MLPs 16->32->32->16, tanh) + N(0,I) prior
    out[b] = mean_n exp(logprob)

Sharding: data-parallel over the batch. Core c handles inputs rows
[256c, 256c+256). Within a core, rows are packed as 16 macro-tiles of
[128 partitions x 512 columns] where partitions = 4 subtiles x 2 groups x 16
features and columns = 2 batch rows x 256 noise rows. The t- and s-MLPs of a
coupling are fused into block-diagonal weights (2 groups at a time), so:
  L1: K=32 -> M=128, 4 subtiles as row-tiled concurrent matmuls
  L2: K=128 -> M=128 dense (block-diagonal content)
  L3: K=128 -> M=32 (t and s separately), 4 subtiles as col-tiled matmuls
All elementwise work (tanh/exp on ScalarE, coupling updates on VectorE) then
runs at full 128-partition width. log|det| accumulates on VectorE; the final
logprob reduction over the 16 feature-partitions per row is a ones-matrix
matmul; mean-over-noise folds into the last exp as a -ln(256) bias plus an
accum_out free-dim sum.
"""

import numpy as np
from contextlib import ExitStack

import concourse.bacc as bacc
import concourse.tile as tile
from concourse import mybir
from concourse.bass_utils import run_bass_kernel_spmd

F32 = mybir.dt.float32
F16 = mybir.dt.float16
AF = mybir.ActivationFunctionType
ALU = mybir.AluOpType

N_CORES = 8
B, NZ, DIM = 2048, 256, 32
HALF, HID = 16, 32
NF = 5
NCPL = 2 * NF              # coupling stages (A/B per flow)
B_CORE = B // N_CORES      # 256
SUBS = 4
GRP = 2
COLS = 512                 # free width per subtile = 2 batch rows x 256 noise
MT = B_CORE // (SUBS * GRP * 2)   # 16 macro-tiles per core (16 b-rows each)

LAST_RESULT = None         # BassKernelResults of the most recent run (for test.py)
REPS = 1                   # benchmarking knob: repeat the whole workload in-program
BUFS = dict(zp=4, hp=4, sp=4, psA=3, psB=2)   # pool sizing knobs
BIGTANH = True             # [128,1024] tanh; PSUM pools resized to fit (psA=3x2+psB=2 banks)
BIGTANH2 = False           # [128,1024] tanh for L2 only (mixed granularity)
ASYM = False               # stream 0 uses big [128,1024] tanh tiles, stream 1 small
ROLLING = False            # rolling two-stream pipeline vs discrete mtile pairs (HW: equivalent)
ILV = 4                    # how many macro-tiles to software-pipeline together
_NC_CACHE = {}             # compiled program cache (program is input-independent)


def _pack_weights(W1, b1, W2, b2, W3, b3):
    """Block-diagonal packed weights, laid out [partition, coupling, free]."""
    w1p = np.zeros((NCPL, 32, 128), np.float32)
    w2p = np.zeros((NCPL, 128, 128), np.float32)
    w3tp = np.zeros((NCPL, 128, 32), np.float32)
    w3sp = np.zeros((NCPL, 128, 32), np.float32)
    b1p = np.zeros((NCPL, 128), np.float32)
    b2p = np.zeros((NCPL, 128), np.float32)
    b3t_vec = np.zeros((128, NCPL), np.float32)
    b3s_vec = np.zeros((128, NCPL), np.float32)
    b3s_total = 0.0
    for i in range(NF):
        for half in range(2):
            c = 2 * i + half
            tn, sn = (0, 1) if half == 0 else (2, 3)
            b3s_total += float(b3[i, sn].sum())
            for g in range(GRP):
                # L1: K = 16g + k  ->  M = 64g + (t: 0-31 | s: 32-63)
                w1p[c, 16 * g:16 * g + 16, 64 * g:64 * g + 32] = W1[i, tn]
                w1p[c, 16 * g:16 * g + 16, 64 * g + 32:64 * g + 64] = W1[i, sn]
                b1p[c, 64 * g:64 * g + 32] = b1[i, tn]
                b1p[c, 64 * g + 32:64 * g + 64] = b1[i, sn]
                # L2: block diagonal on the same hidden layout
                w2p[c, 64 * g:64 * g + 32, 64 * g:64 * g + 32] = W2[i, tn]
                w2p[c, 64 * g + 32:64 * g + 64, 64 * g + 32:64 * g + 64] = W2[i, sn]
                b2p[c, 64 * g:64 * g + 32] = b2[i, tn]
                b2p[c, 64 * g + 32:64 * g + 64] = b2[i, sn]
                # L3: K = hidden -> M = 16g + kk   (t reads t-blocks, s reads s-blocks)
                w3tp[c, 64 * g:64 * g + 32, 16 * g:16 * g + 16] = W3[i, tn]
                w3sp[c, 64 * g + 32:64 * g + 64, 16 * g:16 * g + 16] = W3[i, sn]
            for s in range(SUBS):
                for g in range(GRP):
                    p0 = 32 * s + 16 * g
                    b3t_vec[p0:p0 + 16, c] = b3[i, tn]
                    b3s_vec[p0:p0 + 16, c] = b3[i, sn]

    # SBUF layouts: partition-major, replicated over subtiles where needed
    w1r = np.zeros((128, NCPL, 128), np.float32)
    for s in range(SUBS):
        w1r[32 * s:32 * s + 32] = np.transpose(w1p, (1, 0, 2))
    w1r = w1r.astype(np.float16)
    w2r = np.ascontiguousarray(np.transpose(w2p, (1, 0, 2))).astype(np.float16)
    w3tr = np.ascontiguousarray(np.transpose(w3tp, (1, 0, 2))).astype(np.float16)
    w3sr = np.ascontiguousarray(np.transpose(w3sp, (1, 0, 2))).astype(np.float16)
    # biases tile: columns [b1 | b2 | b3t | b3s | final], each NCPL wide
    biases = np.zeros((128, 4 * NCPL + 1), np.float32)
    biases[:, 0:NCPL] = b1p.T
    biases[:, NCPL:2 * NCPL] = b2p.T
    biases[:, 2 * NCPL:3 * NCPL] = b3t_vec
    biases[:, 3 * NCPL:4 * NCPL] = b3s_vec
    # reduction matrices: cols 0-7 sum 16-partition blocks, cols 8-15 = -0.5x
    red = np.zeros((128, 16), np.float32)
    for p in range(128):
        red[p, p // 16] = 1.0
        red[p, 8 + p // 16] = -0.5
    return w1r, w2r, w3tr, w3sr, biases, red, b3s_total


def _build_program():
    nc = bacc.Bacc("TRN2", target_bir_lowering=False, debug=False,
                   num_devices=N_CORES)
    zl_d = nc.declare_dram_parameter("zl", [MT, 128, COLS], F32, isOutput=False)
    zu_d = nc.declare_dram_parameter("zu", [MT, 128, COLS], F32, isOutput=False)
    w1_d = nc.declare_dram_parameter("w1", [128, NCPL, 128], F16, isOutput=False)
    w2_d = nc.declare_dram_parameter("w2", [128, NCPL, 128], F16, isOutput=False)
    w3t_d = nc.declare_dram_parameter("w3t", [128, NCPL, 32], F16, isOutput=False)
    w3s_d = nc.declare_dram_parameter("w3s", [128, NCPL, 32], F16, isOutput=False)
    bias_d = nc.declare_dram_parameter("biases", [128, 4 * NCPL + 1], F32, isOutput=False)
    red_d = nc.declare_dram_parameter("red", [128, 16], F32, isOutput=False)
    res_d = nc.declare_dram_parameter("res", [8, 2 * MT], F32, isOutput=True)

    with ExitStack() as ctx:
        tc = ctx.enter_context(tile.TileContext(nc))
        wp = ctx.enter_context(tc.tile_pool(name="wp", bufs=1))
        zp = ctx.enter_context(tc.tile_pool(name="zp", bufs=BUFS["zp"]))
        hp = ctx.enter_context(tc.tile_pool(name="hp", bufs=BUFS["hp"]))
        sp = ctx.enter_context(tc.tile_pool(name="sp", bufs=BUFS["sp"]))
        psA = ctx.enter_context(tc.tile_pool(name="psA", bufs=BUFS["psA"], space="PSUM"))
        psB = ctx.enter_context(tc.tile_pool(name="psB", bufs=BUFS["psB"], space="PSUM"))

        w1s = wp.tile([128, NCPL, 128], F16, name="w1s")
        nc.sync.dma_start(w1s[:], w1_d[:])
        w2s = wp.tile([128, NCPL, 128], F16, name="w2s")
        nc.sync.dma_start(w2s[:], w2_d[:])
        w3ts = wp.tile([128, NCPL, 32], F16, name="w3ts")
        nc.sync.dma_start(w3ts[:], w3t_d[:])
        w3ss = wp.tile([128, NCPL, 32], F16, name="w3ss")
        nc.sync.dma_start(w3ss[:], w3s_d[:])
        bia = wp.tile([128, 4 * NCPL + 1], F32, name="bia")
        nc.sync.dma_start(bia[:], bias_d[:])
        red = wp.tile([128, 16], F32, name="red")
        nc.sync.dma_start(red[:], red_d[:])
        res_sb = wp.tile([8, 2 * MT], F32, name="res_sb")

        def load_mtile(rep, mt):
            """DMA + fp16 shadow + state tiles for one macro-tile."""
            st = {}
            st["mt"] = mt
            st["zl"] = zp.tile([128, COLS], F32, tag="zl", name=f"zl{rep}_{mt}")
            nc.sync.dma_start(st["zl"][:], zl_d[mt])
            st["zu"] = zp.tile([128, COLS], F32, tag="zu", name=f"zu{rep}_{mt}")
            nc.sync.dma_start(st["zu"][:], zu_d[mt])
            st["zl16"] = zp.tile([128, COLS], F16, tag="zl16", name=f"zl16_{rep}_{mt}")
            nc.vector.tensor_copy(st["zl16"][:], st["zl"][:])
            st["zu16"] = zp.tile([128, COLS], F16, tag="zu16", name=f"zu16_{rep}_{mt}")
            st["acc"] = sp.tile([128, COLS], F32, tag="acc", name=f"acc{rep}_{mt}")
            return st

        def coupling(rep, st, c, big=False):
            mt = st["mt"]
            inp16, tgt = (st["zl16"], st["zu"]) if c % 2 == 0 else (st["zu16"], st["zl"])
            tgt16 = st["zu16"] if c % 2 == 0 else st["zl16"]
            # L1: 4 row-tiled concurrent matmuls (K=32 strips)
            h1 = hp.tile([128, 2048], F16, tag="h1", name=f"h1_{rep}_{mt}_{c}")
            bc1 = bia[:, c:c + 1]
            if BIGTANH or big:
                for p2 in range(2):
                    h1p = psA.tile([128, 2 * COLS], F32, tag="hidbig", bufs=None,
                                   name=f"h1p{rep}_{mt}_{c}_{p2}")
                    for si in range(2):
                        s = 2 * p2 + si
                        nc.tensor.matmul(
                            h1p[:, 512 * si:512 * si + 512],
                            lhsT=w1s[32 * s:32 * s + 32, c],
                            rhs=inp16[32 * s:32 * s + 32, :], start=True, stop=True,
                            tile_position=(32 * s, 0))
                    nc.scalar.activation(h1[:, 1024 * p2:1024 * p2 + 1024], h1p[:],
                                         AF.Tanh, bias=bc1)
            else:
                for s in range(SUBS):
                    h1p = psA.tile([128, COLS], F32, tag="hid",
                                   name=f"h1p{rep}_{mt}_{c}_{s}")
                    nc.tensor.matmul(
                        h1p[:], lhsT=w1s[32 * s:32 * s + 32, c],
                        rhs=inp16[32 * s:32 * s + 32, :], start=True, stop=True,
                        tile_position=(32 * s, 0))
                    nc.scalar.activation(h1[:, 512 * s:512 * s + 512], h1p[:],
                                         AF.Tanh, bias=bc1)
            # L2: dense 128x128 (block-diagonal content)
            h2 = hp.tile([128, 2048], F16, tag="h2", name=f"h2_{rep}_{mt}_{c}")
            bc2 = bia[:, NCPL + c:NCPL + c + 1]
            if BIGTANH or BIGTANH2 or big:
                for p2 in range(2):
                    h2p = psA.tile([128, 2 * COLS], F32, tag="hidbig", bufs=None,
                                   name=f"h2p{rep}_{mt}_{c}_{p2}")
                    for si in range(2):
                        s = 2 * p2 + si
                        nc.tensor.matmul(
                            h2p[:, 512 * si:512 * si + 512], lhsT=w2s[:, c],
                            rhs=h1[:, 512 * s:512 * s + 512], start=True, stop=True)
                    nc.scalar.activation(h2[:, 1024 * p2:1024 * p2 + 1024], h2p[:],
                                         AF.Tanh, bias=bc2)
            else:
                for s in range(SUBS):
                    h2p = psA.tile([128, COLS], F32, tag="hid",
                                   name=f"h2p{rep}_{mt}_{c}_{s}")
                    nc.tensor.matmul(
                        h2p[:], lhsT=w2s[:, c], rhs=h1[:, 512 * s:512 * s + 512],
                        start=True, stop=True)
                    nc.scalar.activation(h2[:, 512 * s:512 * s + 512], h2p[:],
                                         AF.Tanh, bias=bc2)
            # L3: col-tiled matmuls, 4 subtiles -> partition strips of T/S
            T = psB.tile([128, COLS], F32, tag="ts", name=f"T{rep}_{mt}_{c}")
            S = psB.tile([128, COLS], F32, tag="ts", name=f"S{rep}_{mt}_{c}")
            for s in range(SUBS):
                nc.tensor.matmul(
                    T[32 * s:32 * s + 32, :], lhsT=w3ts[:, c],
                    rhs=h2[:, 512 * s:512 * s + 512], start=True, stop=True,
                    tile_position=(0, 32 * s))
            for s in range(SUBS):
                nc.tensor.matmul(
                    S[32 * s:32 * s + 32, :], lhsT=w3ss[:, c],
                    rhs=h2[:, 512 * s:512 * s + 512], start=True, stop=True,
                    tile_position=(0, 32 * s))
            es = sp.tile([128, COLS], F32, tag="es", name=f"es{rep}_{mt}_{c}")
            nc.scalar.activation(es[:], S[:], AF.Exp,
                                 bias=bia[:, 3 * NCPL + c:3 * NCPL + c + 1])
            # log-det accumulation (raw s; its bias folds into final_bias)
            if c == 0:
                nc.vector.tensor_copy(st["acc"][:], S[:])
            else:
                nc.vector.tensor_add(st["acc"][:], st["acc"][:], S[:])
            # coupling update: tgt = (T + b3t) + tgt * es
            prod = sp.tile([128, COLS], F32, tag="prod", name=f"pr{rep}_{mt}_{c}")
            nc.vector.tensor_mul(prod[:], tgt[:], es[:])
            nc.vector.scalar_tensor_tensor(
                tgt[:], T[:], bia[:, 2 * NCPL + c:2 * NCPL + c + 1], prod[:],
                op0=ALU.add, op1=ALU.add)
            if c + 1 < NCPL:
                nc.vector.tensor_copy(tgt16[:], tgt[:])

        def finish_mtile(rep, st):
            # logprob = sum_k acc - 0.5 sum_k (zl^2 + zu^2) (+ final_bias in exp)
            mt = st["mt"]
            zl, zu = st["zl"], st["zu"]
            sqL = sp.tile([128, COLS], F32, tag="sq", name=f"sqL{rep}_{mt}")
            nc.vector.tensor_mul(sqL[:], zl[:], zl[:])
            sqU = sp.tile([128, COLS], F32, tag="sq2", name=f"sqU{rep}_{mt}")
            nc.vector.tensor_mul(sqU[:], zu[:], zu[:])
            LP = psB.tile([8, COLS], F32, tag="ts", name=f"LP{rep}_{mt}")
            nc.tensor.matmul(LP[:], lhsT=red[:, 0:8], rhs=st["acc"][:],
                             start=True, stop=False, skip_group_check=True)
            nc.tensor.matmul(LP[:], lhsT=red[:, 8:16], rhs=sqL[:],
                             start=False, stop=False, skip_group_check=True)
            nc.tensor.matmul(LP[:], lhsT=red[:, 8:16], rhs=sqU[:],
                             start=False, stop=True, skip_group_check=True)
            # p = exp(logprob + final_bias); accum_out sums the 256 noise cols
            for h in (0, 1):
                pd = sp.tile([8, 256], F32, tag="pd", name=f"pd{rep}_{mt}_{h}")
                nc.scalar.activation(
                    pd[:], LP[:, 256 * h:256 * h + 256], AF.Exp,
                    bias=bia[0:8, 4 * NCPL:4 * NCPL + 1],
                    accum_out=res_sb[:, 2 * mt + h:2 * mt + h + 1])

        # rolling two-stream software pipeline: each engine's static
        # instruction stream alternates between two independent macro-tile
        # chains, and a stream rolls straight into its next macro-tile (with
        # the DMA prefetched early) so there is no drain point between pairs
        def mtile_stream(rep, mts, big=False):
            st = load_mtile(rep, mts[0])
            yield
            for i in range(len(mts)):
                nxt = None
                for c in range(NCPL):
                    coupling(rep, st, c, big=big)
                    yield
                    if c == 1 and i + 1 < len(mts):
                        nxt = load_mtile(rep, mts[i + 1])
                        yield
                finish_mtile(rep, st)
                yield
                st = nxt

        for rep in range(REPS):
            if ROLLING:
                gens = [mtile_stream(rep, list(range(j, MT, ILV)),
                                     big=(ASYM and j == 0))
                        for j in range(ILV)]
                alive = list(gens)
                while alive:
                    for g in list(alive):
                        try:
                            next(g)
                        except StopIteration:
                            alive.remove(g)
            else:
                for mp in range(MT // ILV):
                    sts = [load_mtile(rep, ILV * mp + j) for j in range(ILV)]
                    for c in range(NCPL):
                        for st in sts:
                            coupling(rep, st, c)
                    for st in sts:
                        finish_mtile(rep, st)

        nc.sync.dma_start(res_d[:], res_sb[:])
    nc.compile()
    return nc


def kernel(inputs, noise, W1, b1, W2, b2, W3, b3):
    global LAST_RESULT
    inputs = np.ascontiguousarray(inputs, np.float32)
    noise = np.ascontiguousarray(noise, np.float32)
    assert inputs.shape == (B, DIM) and noise.shape == (NZ, DIM)

    w1r, w2r, w3tr, w3sr, biases, red, b3s_total = _pack_weights(
        np.asarray(W1), np.asarray(b1), np.asarray(W2), np.asarray(b2),
        np.asarray(W3), np.asarray(b3))
    final_bias = float(-0.5 * DIM * np.log(2.0 * np.pi) - np.log(NZ) + b3s_total)
    biases[:, 4 * NCPL] = final_bias

    # Host-side z construction in the exact SBUF layout:
    # [core, mt, (s,g,k), (h,n)] with b = ((((c*16+mt)*4+s)*2+g)*2+h
    zfull = inputs[:, None, :] - noise[None, :, :]            # [B, NZ, 32]
    z6 = zfull.reshape(N_CORES, MT, SUBS, GRP, 2, NZ, DIM)
    zl_all = np.ascontiguousarray(
        z6[..., :HALF].transpose(0, 1, 2, 3, 6, 4, 5).reshape(N_CORES, MT, 128, COLS))
    zu_all = np.ascontiguousarray(
        z6[..., HALF:].transpose(0, 1, 2, 3, 6, 4, 5).reshape(N_CORES, MT, 128, COLS))

    key = (MT, REPS, ILV)
    if key not in _NC_CACHE:
        _NC_CACHE[key] = _build_program()
    nc = _NC_CACHE[key]
    in_maps = [
        {"zl": zl_all[c], "zu": zu_all[c], "w1": w1r, "w2": w2r,
         "w3t": w3tr, "w3s": w3sr, "biases": biases, "red": red}
        for c in range(N_CORES)
    ]
    br = run_bass_kernel_spmd(nc, in_maps, list(range(N_CORES)))
    LAST_RESULT = br

    outs = []
    for c in range(N_CORES):
        res = np.asarray(br.results[c]["res"])                 # [8, 2*MT]
        outs.append(res.reshape(8, MT, 2).transpose(1, 0, 2).reshape(B_CORE))
    return np.concatenate(outs).astype(np.float32)

